# revision 1
# baseline (speedup 1.0000x reference)
"""Cosine-similarity multi-head attention on 8 TRN2 NeuronCores.

Problem: B=4, N=2048, E=1024, H=16, D=64.
Sharding: core c handles batch b=c//2 and head-group g=c%2 (8 heads, 512
model cols). Each core computes its heads' attention and a partial output
projection; the host sums the two partials per batch and adds the folded
output bias.

Device-side layout: everything is computed transposed.
  xT [E, N] (host pre-transposes) ->
  qT/kT = W.T @ xT   [m, n]  (heads on partitions, tokens on free dim)
  v    = xT.T @ Wv   [n, m]  (natural)
  S^T[j, i] = kn_j . qn_i    (keys on partitions)
  outT[d, i] = sum_j v[j, d] exp(S^T[j, i])  (+ row 64 = softmax denom via
                                              a ones column in v)
  yT[eo, n] = sum_m Wo[m, eo] outT[m, n]     (transposed, DMA'd out)

DMA throughput in this environment is descriptor-count-bound (~160ns per
descriptor, one descriptor per partition-contiguous run), so all DRAM
tensors use partition-major host layouts: every partition's whole payload
is one contiguous run (x: 128 descriptors of 32KB instead of 1024 of 4KB).
Constants are built on-device with memset/affine_select instead of DMA.
"""

import sys

sys.path.insert(0, "/opt/trn_rl_repo")

import numpy as np
import ml_dtypes

B, N, E, H = 4, 2048, 1024, 16
D = E // H           # 64
M_CORES = 8
HC = H // 2          # heads per core = 8
EC = E // 2          # model cols per core = 512
ET = E // 128        # 8 e-tiles
NT = N // 128        # 16 n-tiles
MT = EC // 128       # 4 m-tiles (head pairs)
JT = N // 128        # 16 key tiles
BF16 = ml_dtypes.bfloat16

_CACHE = {}


def build_nc(repeat=1, variant="full"):
    """Build + finalize the single-core Bass program (same on all cores).

    repeat>1 duplicates the whole computation serially inside one NEFF —
    used by the bench harness to measure per-iteration time above the
    ~100ms axon dispatch overhead."""
    key = ("nc", repeat, variant)
    if key in _CACHE:
        return _CACHE[key]
    import concourse.bass as bass  # noqa: F401
    from concourse import bacc
    import concourse.mybir as mybir
    import concourse.tile as tile
    from concourse.masks import make_identity
    from contextlib import ExitStack

    f32 = mybir.dt.float32
    bf16 = mybir.dt.bfloat16
    AF = mybir.ActivationFunctionType

    # Make Exp and Ln resolve to the combined natural_log_exp table set so
    # the act-table-load pass doesn't ping-pong between exp_and_others and
    # natural_log on every projection/attention transition. Positions in the
    # table list are load-bearing (index == act_func_set_id), so only the
    # function sets are filtered.
    if not getattr(bacc, "_act_tables_patched", False):
        _orig_gat = bacc.get_activation_tables

        def _gat(arch):
            t = dict(_orig_gat(arch))
            for k in t:
                if k != "natural_log_exp_and_others":
                    t[k] = {
                        f for f in t[k]
                        if str(f).split(".")[-1] not in ("Exp", "Ln")
                    }
            return t

        bacc.get_activation_tables = _gat
        bacc._act_tables_patched = True

    nc = bacc.Bacc()
    # All inputs partition-major: [128, ...] with everything one partition
    # needs contiguous along the trailing dims.
    xT = nc.declare_dram_parameter("xT", [128, ET, N], bf16, isOutput=False)
    wq = nc.declare_dram_parameter("wq", [128, ET, EC], bf16, isOutput=False)
    wk = nc.declare_dram_parameter("wk", [128, ET, EC], bf16, isOutput=False)
    wv = nc.declare_dram_parameter("wv", [128, ET, EC], bf16, isOutput=False)
    wo = nc.declare_dram_parameter("wo", [128, MT, E], bf16, isOutput=False)
    qkb8 = nc.declare_dram_parameter("qkb8", [8, 128], f32, isOutput=False)
    y = nc.declare_dram_parameter("y", [128, ET, N], f32, isOutput=True)

    with tile.TileContext(nc) as tc:
      for _rep in range(repeat):
        with ExitStack() as ctx:
            cpool = ctx.enter_context(tc.sbuf_pool(name="consts", bufs=1))
            wqkv = ctx.enter_context(tc.sbuf_pool(name="wqkv", bufs=1))
            wop = ctx.enter_context(tc.sbuf_pool(name="wo", bufs=1))
            xp = ctx.enter_context(tc.sbuf_pool(name="xT", bufs=1))
            qkp = ctx.enter_context(tc.sbuf_pool(name="qkv", bufs=1))
            otp = ctx.enter_context(tc.sbuf_pool(name="outT", bufs=1))
            stg = ctx.enter_context(tc.sbuf_pool(name="stg", bufs=2))
            ep = ctx.enter_context(tc.sbuf_pool(name="exp", bufs=3))
            yp = ctx.enter_context(tc.sbuf_pool(name="y", bufs=2))
            # PSUM: pp 2 banks + s 2x2 banks + pv 2 banks = 8
            pp = ctx.enter_context(tc.psum_pool(name="pp", bufs=2))
            sp = ctx.enter_context(tc.psum_pool(name="sp", bufs=2))
            pvp = ctx.enter_context(tc.psum_pool(name="pvp", bufs=1))

            # ---- big input DMAs (one per tensor; 128 fat descriptors) ----
            xbig = xp.tile([128, ET * N], bf16, tag="xbig", name="xbig")
            nc.sync.dma_start(
                xbig[:].rearrange("p (e n) -> p e n", e=ET), xT[:, :, :]
            )
            x_t = [xbig[:, et * N:(et + 1) * N] for et in range(ET)]
            w_t = {}
            for nm, drh in (("q", wq), ("k", wk), ("v", wv)):
                wbig = wqkv.tile([128, ET * EC], bf16, tag=f"wb{nm}",
                                 name=f"wb{nm}")
                nc.scalar.dma_start(
                    wbig[:].rearrange("p (e n) -> p e n", e=ET), drh[:, :, :]
                )
                for et in range(ET):
                    w_t[nm, et] = wbig[:, et * EC:(et + 1) * EC]
            wobig = wop.tile([128, MT * E], bf16, tag="wob", name="wob")
            nc.scalar.dma_start(
                wobig[:].rearrange("p (m n) -> p m n", m=MT), wo[:, :, :]
            )
            wo_t = [wobig[:, mt * E:(mt + 1) * E] for mt in range(MT)]

            # ---- constants built on device ----
            # qkb8 [8, 128] -> [128, 8] via identity matmul transpose
            qkb_t = cpool.tile([128, 8], f32, tag="qkb", name="qkb")
            if variant in ("c1", "c2"):
                nc.vector.memset(qkb_t[:], 0.0)
            else:
                qkb8_t = cpool.tile([8, 128], f32, tag="qkb8", name="qkb8")
                nc.sync.dma_start(qkb8_t[:], qkb8[:, :])
                id8 = cpool.tile([8, 8], f32, tag="id8", name="id8")
                make_identity(nc, id8[:])
                qkb_ps = pp.tile([128, 8], f32, tag="pp", name="qkb_ps")
                nc.tensor.matmul(qkb_ps[:], lhsT=qkb8_t[:], rhs=id8[:],
                                 start=True, stop=True)
                nc.vector.tensor_copy(qkb_t[:], qkb_ps[:])
            # masks: per-parity column sums / broadcasts
            msum_t = cpool.tile([128, 2], bf16, tag="msum", name="msum")
            nc.vector.memset(msum_t[:], 0.0)
            nc.vector.memset(msum_t[0:64, 0:1], 1.0)
            nc.vector.memset(msum_t[64:128, 1:2], 1.0)
            # mbc[p, f] = 1 iff 0 <= f - 64p < 64 (partition-1-offset memsets
            # are rejected by the BIR verifier, so build via affine_select)
            mbc_t = cpool.tile([2, 128], bf16, tag="mbc", name="mbc")
            nc.vector.memset(mbc_t[:], 1.0)
            nc.gpsimd.affine_select(
                out=mbc_t[:], in_=mbc_t[:],
                compare_op=mybir.AluOpType.is_ge, fill=0.0,
                base=0, pattern=[[1, 128]], channel_multiplier=-64)
            nc.gpsimd.affine_select(
                out=mbc_t[:], in_=mbc_t[:],
                compare_op=mybir.AluOpType.is_ge, fill=0.0,
                base=63, pattern=[[-1, 128]], channel_multiplier=64)
            ones_t = cpool.tile([1, 64], f32, tag="ones1", name="ones1")
            nc.vector.memset(ones_t[:], 1.0)
            eps_t = cpool.tile([2, 1], f32, tag="eps", name="eps")
            nc.vector.memset(eps_t[:], 1e-12)

            # persistent activations
            qn_t = [qkp.tile([128, N], bf16, tag=f"qn{mt}", name=f"qn{mt}")
                    for mt in range(MT)]
            kn_t = [qkp.tile([128, N], bf16, tag=f"kn{mt}", name=f"kn{mt}")
                    for mt in range(MT)]
            v_t = [qkp.tile([128, HC * (D + 1)], bf16, tag=f"v{nt}",
                            name=f"v{nt}") for nt in range(NT)]
            outT_t = [otp.tile([128, N], bf16, tag=f"ot{mt}", name=f"ot{mt}")
                      for mt in range(MT)]

            # ---- V projection (natural layout [n, m]) + ones column ----
            for nt in range(0 if variant in ("dmaonly", "c1") else NT):
                ps = pp.tile([128, 512], f32, tag="pp", name="pp")
                for et in range(ET):
                    nc.tensor.matmul(
                        ps[:],
                        lhsT=x_t[et][:, nt * 128:(nt + 1) * 128],
                        rhs=w_t["v", et],
                        start=(et == 0),
                        stop=(et == ET - 1),
                    )
                vt = v_t[nt]
                v3 = vt[:].rearrange("p (h e) -> p h e", h=HC)
                nc.vector.tensor_copy(
                    v3[:, :, 0:D], ps[:].rearrange("p (h d) -> p h d", h=HC)
                )
                nc.vector.memset(v3[:, :, D:D + 1], 1.0)

            # ---- Q/K projections + l2 normalization ([m, n] layout) ----
            def qk_proj(mt, nm, dst, bias_col):
                qf = stg.tile([128, N], f32, tag="qf", name="qf")
                for ch in range(4):
                    ps = pp.tile([128, 512], f32, tag="pp", name="pp")
                    for et in range(ET):
                        nc.tensor.matmul(
                            ps[:],
                            lhsT=w_t[nm, et][:, mt * 128:(mt + 1) * 128],
                            rhs=x_t[et][:, ch * 512:(ch + 1) * 512],
                            start=(et == 0),
                            stop=(et == ET - 1),
                        )
                    nc.vector.tensor_scalar_add(
                        qf[:, ch * 512:(ch + 1) * 512], ps[:],
                        qkb_t[:, bias_col:bias_col + 1],
                    )
                rcp = stg.tile([2, N], bf16, tag="rcp", name="rcp", bufs=2)
                for ch in range(4):
                    sq = stg.tile([128, 512], bf16, tag="sq", name="sq")
                    nc.vector.tensor_mul(
                        sq[:], qf[:, ch * 512:(ch + 1) * 512],
                        qf[:, ch * 512:(ch + 1) * 512])
                    nps = pp.tile([2, 512], f32, tag="pp", name="npp")
                    nc.tensor.matmul(nps[:], lhsT=msum_t[:], rhs=sq[:],
                                     start=True, stop=True)
                    rs = stg.tile([2, 512], f32, tag="rs", name="rs")
                    # 1/sqrt(s+eps) = exp(-0.5*ln(s+eps)); Ln+Exp share one
                    # activation table set (sqrt's is separate and would
                    # thrash), and beat sqrt's 65536-ULP budget.
                    nc.scalar.activation(rs[:], nps[:], AF.Ln, bias=eps_t[:])
                    nc.scalar.activation(
                        rcp[:, ch * 512:(ch + 1) * 512], rs[:], AF.Exp,
                        scale=-0.5)
                for ch in range(4):
                    bc = pp.tile([128, 512], f32, tag="pp", name="bc")
                    nc.tensor.matmul(
                        bc[:], lhsT=mbc_t[:],
                        rhs=rcp[:, ch * 512:(ch + 1) * 512],
                        start=True, stop=True,
                    )
                    nc.vector.tensor_mul(
                        dst[:, ch * 512:(ch + 1) * 512],
                        qf[:, ch * 512:(ch + 1) * 512], bc[:],
                    )

            # ---- attention for one head ----
            def attend(h):
                mt, par = h // 2, h % 2
                pr = 64 * par
                for ic2 in range(2):
                    i0 = ic2 * 1024
                    pv = pvp.tile([65, 1024], f32, tag="pv", name="pv")
                    for jt in range(JT):
                        s = sp.tile([128, 1024], f32, tag="s", name="s")
                        for c in range(2):
                            nc.tensor.matmul(
                                s[:, c * 512:(c + 1) * 512],
                                lhsT=kn_t[mt][pr:pr + 64,
                                              jt * 128:(jt + 1) * 128],
                                rhs=qn_t[mt][pr:pr + 64,
                                             i0 + c * 512:i0 + (c + 1) * 512],
                                start=True, stop=True,
                            )
                        e = ep.tile([128, 1024], bf16, tag="e", name="e")
                        if variant == "noexp":
                            nc.gpsimd.memset(e[:], 1.0)
                        else:
                            nc.scalar.activation(e[:], s[:], AF.Exp)
                        for c in range(2):
                            nc.tensor.matmul(
                                pv[:, c * 512:(c + 1) * 512],
                                lhsT=v_t[jt][:, h * (D + 1):(h + 1) * (D + 1)],
                                rhs=e[:, c * 512:(c + 1) * 512],
                                start=(jt == 0), stop=(jt == JT - 1),
                            )
                    rcd = stg.tile([1, 1024], f32, tag="rcd", name="rcd",
                                   bufs=1)
                    nc.vector.reciprocal(rcd[:], pv[64:65, :])
                    bc = sp.tile([64, 1024], f32, tag="s", name="bcd")
                    for c in range(2):
                        nc.tensor.matmul(
                            bc[:, c * 512:(c + 1) * 512],
                            lhsT=ones_t[:],
                            rhs=rcd[:, c * 512:(c + 1) * 512],
                            start=True, stop=True,
                        )
                    bcs = stg.tile([64, 1024], f32, tag="bcs", name="bcs",
                                   bufs=1)
                    nc.vector.tensor_copy(bcs[:], bc[:])
                    nc.vector.tensor_mul(
                        outT_t[mt][pr:pr + 64, i0:i0 + 1024], pv[0:64, :],
                        bcs[:]
                    )

            # interleave: projections of pair mt feed heads 2mt, 2mt+1 while
            # the (ACT-heavy) attention of the previous pair still runs.
            for mt in range(MT):
                if variant in ("dmaonly", "c1"):
                    break
                qk_proj(mt, "q", qn_t[mt], mt)
                qk_proj(mt, "k", kn_t[mt], 4 + mt)
                attend(2 * mt)
                attend(2 * mt + 1)

            # ---- output projection, transposed: yT[eo, n] ----
            for et in range(1 if variant in ("dmaonly", "c1") else ET):
                yt = yp.tile([128, N], f32, tag="y", name="y")
                for ch in range(4):
                    ps = pp.tile([128, 512], f32, tag="pp", name="yps")
                    for mt in range(MT):
                        nc.tensor.matmul(
                            ps[:],
                            lhsT=wo_t[mt][:, et * 128:(et + 1) * 128],
                            rhs=outT_t[mt][:, ch * 512:(ch + 1) * 512],
                            start=(mt == 0), stop=(mt == MT - 1),
                        )
                    nc.vector.tensor_copy(yt[:, ch * 512:(ch + 1) * 512],
                                          ps[:])
                nc.sync.dma_start(y[:, et, :], yt[:])

    nc.finalize()
    _CACHE[key] = nc
    return nc


def make_in_maps(x, Wq_w, Wq_b, Wk_w, Wk_b, Wv_w, Wv_b, Wo_w, Wo_b):
    x = np.asarray(x, dtype=np.float32)

    def pmajor(a, tiles):
        # [tiles*128, F] -> [128, tiles, F] (partition-major)
        return np.ascontiguousarray(
            a.reshape(tiles, 128, a.shape[1]).transpose(1, 0, 2)
        ).astype(BF16)

    in_maps = []
    for c in range(M_CORES):
        b, g = c // 2, c % 2
        cols = slice(g * EC, (g + 1) * EC)
        qb = np.asarray(Wq_b, np.float32)[cols].reshape(MT, 128)
        kb = np.asarray(Wk_b, np.float32)[cols].reshape(MT, 128)
        qkb8 = np.zeros((8, 128), np.float32)
        qkb8[0:MT] = qb
        qkb8[4:4 + MT] = kb
        in_maps.append({
            "xT": pmajor(np.ascontiguousarray(x[b].T), ET),
            "wq": pmajor(np.asarray(Wq_w, np.float32)[:, cols], ET),
            "wk": pmajor(np.asarray(Wk_w, np.float32)[:, cols], ET),
            "wv": pmajor(np.asarray(Wv_w, np.float32)[:, cols], ET),
            "wo": pmajor(np.asarray(Wo_w, np.float32)[cols, :], MT),
            "qkb8": qkb8,
        })
    return in_maps


def assemble(results, Wv_b, Wo_w, Wo_b):
    bias_eff = (np.asarray(Wv_b, np.float32) @ np.asarray(Wo_w, np.float32)
                + np.asarray(Wo_b, np.float32))
    out = np.empty((B, N, E), np.float32)
    for b in range(B):
        # y is [128, ET, N] partition-major of yT [E, N]
        yT = results[2 * b]["y"] + results[2 * b + 1]["y"]
        yT = yT.transpose(1, 0, 2).reshape(E, N)
        out[b] = yT.T + bias_eff
    return out


def kernel(x, Wq_w, Wq_b, Wk_w, Wk_b, Wv_w, Wv_b, Wo_w, Wo_b):
    from concourse.bass_utils import run_bass_kernel_spmd

    nc = build_nc()
    in_maps = make_in_maps(x, Wq_w, Wq_b, Wk_w, Wk_b, Wv_w, Wv_b, Wo_w, Wo_b)
    res = run_bass_kernel_spmd(nc, in_maps, list(range(M_CORES)))
    return assemble(res.results, Wv_b, Wo_w, Wo_b)



# revision 5
# speedup vs baseline: 1.1768x; 1.1768x over previous
"""Cosine-similarity multi-head attention on 8 TRN2 NeuronCores.

Problem: B=4, N=2048, E=1024, H=16, D=64.
Sharding: core c handles batch b=c//2 and head-group g=c%2 (8 heads, 512
model cols). Each core computes its heads' attention and a partial output
projection; the host sums the two partials per batch and adds the folded
output bias.

Device-side layout (v2):
  xT [E, N] fp8 (host pre-transposes, fp8e4) ->
  qT/kT = W.T @ xT   [m, n]  via fp8 DoubleRow matmuls (2 k-tiles/instr)
  v    = xT.T @ Wv   [n, m]  fp8 DoubleRow; stored [128, nt, h, d+1] bf16
                             with a ones column for the softmax denominator
  S^T[j, i] = kn_j . qn_i    bf16 (keys on partitions)
  e = exp(S^T) on ACT, or via the bf16-bitcast Schraudolph trick on
      DVE/Pool (one tensor_scalar: bits = A*s + B, int16 out, viewed bf16)
  PV TRANSPOSED: out[i, d] accumulated as e_blk.T @ v (full 128-part out,
      F=65) -> per-token denominator lands per-PARTITION, so the Pool
      engine's normalize_recip does the whole softmax division.
  outT[d, i] recovered via PE transposes, then yT = Wo.T @ outT as before.

All DRAM tensors use partition-major host layouts (DMA is descriptor-count
bound). Weights are pre-scaled x16 on the host for fp8 range (the q/k scale
cancels in l2-normalization; the v scale is folded into Wo/16).
"""

import sys

sys.path.insert(0, "/opt/trn_rl_repo")

import numpy as np
import ml_dtypes

B, N, E, H = 4, 2048, 1024, 16
D = E // H           # 64
M_CORES = 8
HC = H // 2          # heads per core = 8
EC = E // 2          # model cols per core = 512
ET = E // 128        # 8 e-tiles
NT = N // 128        # 16 n-tiles
MT = EC // 128       # 4 m-tiles (head pairs)
JT = N // 128        # 16 key tiles
BF16 = ml_dtypes.bfloat16
FP8 = ml_dtypes.float8_e4m3

# --- optimization toggles ---
PROJ_FP8 = False       # QKV projections via fp8e4 DoubleRow matmuls (too
                       # coarse: rel err 4e-2 measured; keep off)
PV_TRANSPOSED = True   # PV as e^T @ v with Pool normalize_recip
W_SCALE = 16.0         # host-side weight scale for fp8 range

# custom-DVE exp: exp(x) ~= ((x + C0) * (x*C1) + C2)^16 for |x| <= 1.05,
# max rel err 2.2e-4 in f32 (the bf16 e-tile quantization of 4e-3 dominates,
# identically to the ACT-exp path). 7 of 8 DVE ALU stages.
EXP_C0 = 32.02436713560685
EXP_C1 = 0.00195263
EXP_C2 = 1.00000051

# exp engine schedule: cycle over this string per score tile (256 per rep).
# 'a' = ACT table exp, 'd' = DVE custom-op exp.
EXP_CYCLE = "aad"

# Pool (gpsimd) offload of PSUM-reading/writing data movement. The cost
# model allows Pool<->PSUM; flip off if HW disagrees.
POOL_MEMSET = True     # pvt accumulator zeroing
POOL_OSB = True        # pvt psum -> sbuf copy before normalize_recip
POOL_YT = True         # output-projection psum -> sbuf copies

_CACHE = {}


def _register_exp_op():
    """Register the EXP_POLY16_ANT custom-DVE op (idempotent)."""
    from concourse import dve_ops
    from concourse.dve_spec import Spec, Src0, C0, C1, C2, sq, lower
    from concourse.dve_uop import DveOpSpec

    name = "EXP_POLY16_ANT"
    for op in dve_ops.OPS:
        if op.name == name:
            return op

    y = (Src0 + C0) * (Src0 * C1) + C2
    e = sq(sq(sq(sq(y))))

    def ref(in0, in1, s0, s1, imm2):
        x = np.asarray(in0, np.float32)
        a = np.float32(s0 if not isinstance(s0, np.ndarray) else s0)
        b = np.float32(s1 if not isinstance(s1, np.ndarray) else s1)
        yv = ((x + a) * (x * b) + np.float32(imm2)).astype(np.float32)
        for _ in range(4):
            yv = (yv * yv).astype(np.float32)
        return yv

    spec = Spec(body=e, reference=ref)
    row = max(dve_ops._SUB_OPCODE_FOR_NAME.values()) + 1
    assert row < 0x20
    dve_ops._SUB_OPCODE_FOR_NAME[name] = row
    op = dve_ops.DveOp(name, spec, subdim=False, uops_sha={})
    shas = {}
    for ver in ("v3", "v4"):
        d = DveOpSpec(name=name, opcode=row, uops=lower(spec, ver=ver),
                      rd1_en=False)
        shas[ver] = d.sha(ver)
    object.__setattr__(op, "uops_sha", shas)
    dve_ops.OPS.append(op)
    dve_ops.CUSTOM_DVE_SPECS[name] = spec
    return op


def build_nc(repeat=1, variant="full"):
    """Build + finalize the single-core Bass program (same on all cores).

    repeat>1 duplicates the whole computation serially inside one NEFF —
    used by the bench harness to measure per-iteration time above the
    ~100ms axon dispatch overhead."""
    key = ("nc", repeat, variant, PROJ_FP8, PV_TRANSPOSED, EXP_CYCLE)
    if key in _CACHE:
        return _CACHE[key]
    import concourse.bass as bass  # noqa: F401
    from concourse import bacc
    import concourse.mybir as mybir
    import concourse.tile as tile
    from concourse.masks import make_identity
    from contextlib import ExitStack

    f32 = mybir.dt.float32
    bf16 = mybir.dt.bfloat16
    fp8 = mybir.dt.float8e4
    i16 = mybir.dt.int16
    AF = mybir.ActivationFunctionType
    DR = mybir.MatmulPerfMode.DoubleRow
    ALU = mybir.AluOpType

    # Make Exp and Ln resolve to the combined natural_log_exp table set so
    # the act-table-load pass doesn't ping-pong between exp_and_others and
    # natural_log on every projection/attention transition. Positions in the
    # table list are load-bearing (index == act_func_set_id), so only the
    # function sets are filtered.
    if not getattr(bacc, "_act_tables_patched", False):
        _orig_gat = bacc.get_activation_tables

        def _gat(arch):
            t = dict(_orig_gat(arch))
            for k in t:
                if k != "natural_log_exp_and_others":
                    t[k] = {
                        f for f in t[k]
                        if str(f).split(".")[-1] not in ("Exp", "Ln")
                    }
            return t

        bacc.get_activation_tables = _gat
        bacc._act_tables_patched = True

    xdt = fp8 if PROJ_FP8 else bf16

    nc = bacc.Bacc()
    # All inputs partition-major: [128, ...] with everything one partition
    # needs contiguous along the trailing dims.
    xT = nc.declare_dram_parameter("xT", [128, ET, N], xdt, isOutput=False)
    wq = nc.declare_dram_parameter("wq", [128, ET, EC], xdt, isOutput=False)
    wk = nc.declare_dram_parameter("wk", [128, ET, EC], xdt, isOutput=False)
    wv = nc.declare_dram_parameter("wv", [128, ET, EC], xdt, isOutput=False)
    wo = nc.declare_dram_parameter("wo", [128, MT, E], bf16, isOutput=False)
    qkb8 = nc.declare_dram_parameter("qkb8", [8, 128], f32, isOutput=False)
    y = nc.declare_dram_parameter("y", [128, ET, N], f32, isOutput=True)

    exp_idx = [0]

    with tile.TileContext(nc) as tc:
      for _rep in range(repeat):
        with ExitStack() as ctx:
            cpool = ctx.enter_context(tc.sbuf_pool(name="consts", bufs=1))
            wqkv = ctx.enter_context(tc.sbuf_pool(name="wqkv", bufs=1))
            wop = ctx.enter_context(tc.sbuf_pool(name="wo", bufs=1))
            xp = ctx.enter_context(tc.sbuf_pool(name="xT", bufs=1))
            qkp = ctx.enter_context(tc.sbuf_pool(name="qkv", bufs=1))
            vp = ctx.enter_context(tc.sbuf_pool(name="vv", bufs=1))
            otp = ctx.enter_context(tc.sbuf_pool(name="outT", bufs=1))
            stg = ctx.enter_context(tc.sbuf_pool(name="stg", bufs=2))
            ep = ctx.enter_context(tc.sbuf_pool(name="exp", bufs=3))
            osb = ctx.enter_context(tc.sbuf_pool(name="osb", bufs=2))
            yp = ctx.enter_context(tc.sbuf_pool(name="y", bufs=2))
            # PSUM banks: pp 2 + s 2x2 + pvt 1x2 = 8
            pp = ctx.enter_context(tc.psum_pool(name="pp", bufs=2))
            sp = ctx.enter_context(tc.psum_pool(name="sp", bufs=2))
            pvp = ctx.enter_context(tc.psum_pool(name="pvp", bufs=1))

            # ---- big input DMAs (one per tensor; 128 fat descriptors) ----
            xbig = xp.tile([128, ET * N], xdt, tag="xbig", name="xbig")
            nc.sync.dma_start(
                xbig[:].rearrange("p (e n) -> p e n", e=ET), xT[:, :, :]
            )
            xr = xbig[:].rearrange("p (e n) -> p e n", e=ET)
            x_t = [xbig[:, et * N:(et + 1) * N] for et in range(ET)]
            w_t = {}
            wr = {}
            for nm, drh in (("q", wq), ("k", wk), ("v", wv)):
                wbig = wqkv.tile([128, ET * EC], xdt, tag=f"wb{nm}",
                                 name=f"wb{nm}")
                nc.scalar.dma_start(
                    wbig[:].rearrange("p (e n) -> p e n", e=ET), drh[:, :, :]
                )
                wr[nm] = wbig[:].rearrange("p (e n) -> p e n", e=ET)
                for et in range(ET):
                    w_t[nm, et] = wbig[:, et * EC:(et + 1) * EC]
            wobig = wop.tile([128, MT * E], bf16, tag="wob", name="wob")
            nc.scalar.dma_start(
                wobig[:].rearrange("p (m n) -> p m n", m=MT), wo[:, :, :]
            )
            wo_t = [wobig[:, mt * E:(mt + 1) * E] for mt in range(MT)]

            # ---- constants built on device ----
            # qkb8 [8, 128] -> [128, 8] via identity matmul transpose
            qkb_t = cpool.tile([128, 8], f32, tag="qkb", name="qkb")
            if variant in ("c1", "c2"):
                nc.vector.memset(qkb_t[:], 0.0)
            else:
                qkb8_t = cpool.tile([8, 128], f32, tag="qkb8", name="qkb8")
                nc.sync.dma_start(qkb8_t[:], qkb8[:, :])
                id8 = cpool.tile([8, 8], f32, tag="id8", name="id8")
                make_identity(nc, id8[:])
                qkb_ps = pp.tile([128, 8], f32, tag="pp", name="qkb_ps")
                nc.tensor.matmul(qkb_ps[:], lhsT=qkb8_t[:], rhs=id8[:],
                                 start=True, stop=True)
                nc.vector.tensor_copy(qkb_t[:], qkb_ps[:])
            # masks: per-parity column sums / broadcasts
            msum_t = cpool.tile([128, 2], bf16, tag="msum", name="msum")
            nc.vector.memset(msum_t[:], 0.0)
            nc.vector.memset(msum_t[0:64, 0:1], 1.0)
            nc.vector.memset(msum_t[64:128, 1:2], 1.0)
            # mbc[p, f] = 1 iff 0 <= f - 64p < 64 (partition-1-offset memsets
            # are rejected by the BIR verifier, so build via affine_select)
            mbc_t = cpool.tile([2, 128], bf16, tag="mbc", name="mbc")
            nc.vector.memset(mbc_t[:], 1.0)
            nc.gpsimd.affine_select(
                out=mbc_t[:], in_=mbc_t[:],
                compare_op=ALU.is_ge, fill=0.0,
                base=0, pattern=[[1, 128]], channel_multiplier=-64)
            nc.gpsimd.affine_select(
                out=mbc_t[:], in_=mbc_t[:],
                compare_op=ALU.is_ge, fill=0.0,
                base=63, pattern=[[-1, 128]], channel_multiplier=64)
            eps_t = cpool.tile([2, 1], f32, tag="eps", name="eps")
            nc.vector.memset(eps_t[:], 1e-12)
            id128 = cpool.tile([128, 128], bf16, tag="id128", name="id128")
            make_identity(nc, id128[:])

            # persistent activations
            qn_t = [qkp.tile([128, N], bf16, tag=f"qn{mt}", name=f"qn{mt}")
                    for mt in range(MT)]
            kn_t = [qkp.tile([128, N], bf16, tag=f"kn{mt}", name=f"kn{mt}")
                    for mt in range(MT)]
            # v: [128, nt, h, d+1] bf16, ones column at d=64
            vbig = vp.tile([128, NT * HC * (D + 1)], bf16, tag="vbig",
                           name="vbig")
            v4 = vbig[:].rearrange("p (nt h d) -> p nt h d", nt=NT, h=HC)
            outT_t = [otp.tile([128, N], bf16, tag=f"ot{mt}", name=f"ot{mt}")
                      for mt in range(MT)]

            def proj_matmuls(ps, nm, outcols, rhs_ap):
                """Accumulate W[:, outcols].T @ rhs into ps."""
                if PROJ_FP8:
                    for etp in range(ET // 2):
                        nc.tensor.matmul(
                            ps,
                            lhsT=wr[nm][:, 2 * etp:2 * etp + 2, outcols],
                            rhs=rhs_ap(2 * etp),
                            start=(etp == 0), stop=(etp == ET // 2 - 1),
                            perf_mode=DR,
                        )
                else:
                    for et in range(ET):
                        nc.tensor.matmul(
                            ps,
                            lhsT=w_t[nm, et][:, outcols],
                            rhs=rhs_ap(et),
                            start=(et == 0), stop=(et == ET - 1),
                        )

            # ---- V projection (natural layout [n, m]) + ones column ----
            for nt in range(0 if variant in ("dmaonly", "c1") else NT):
                ps = pp.tile([128, 512], f32, tag="pp", name="pp")
                if PROJ_FP8:
                    for etp in range(ET // 2):
                        nc.tensor.matmul(
                            ps[:],
                            lhsT=xr[:, 2 * etp:2 * etp + 2,
                                    nt * 128:(nt + 1) * 128],
                            rhs=wr["v"][:, 2 * etp:2 * etp + 2, :],
                            start=(etp == 0), stop=(etp == ET // 2 - 1),
                            perf_mode=DR,
                        )
                else:
                    for et in range(ET):
                        nc.tensor.matmul(
                            ps[:],
                            lhsT=x_t[et][:, nt * 128:(nt + 1) * 128],
                            rhs=w_t["v", et],
                            start=(et == 0), stop=(et == ET - 1),
                        )
                nc.vector.tensor_copy(
                    v4[:, nt, :, 0:D],
                    ps[:].rearrange("p (h d) -> p h d", h=HC),
                )
                nc.vector.memset(v4[:, nt, :, D:D + 1], 1.0)

            # ---- Q/K projections + l2 normalization ([m, n] layout) ----
            def qk_proj(mt, nm, dst, bias_col):
                qf = stg.tile([128, N], f32, tag="qf", name="qf")
                for ch in range(4):
                    ps = pp.tile([128, 512], f32, tag="pp", name="pp")
                    proj_matmuls(
                        ps[:], nm, slice(mt * 128, (mt + 1) * 128),
                        lambda et: xr[:, et:et + 2, ch * 512:(ch + 1) * 512]
                        if PROJ_FP8 else x_t[et][:, ch * 512:(ch + 1) * 512],
                    )
                    nc.vector.tensor_scalar_add(
                        qf[:, ch * 512:(ch + 1) * 512], ps[:],
                        qkb_t[:, bias_col:bias_col + 1],
                    )
                rcp = stg.tile([2, N], bf16, tag="rcp", name="rcp", bufs=2)
                for ch in range(4):
                    sq = stg.tile([128, 512], bf16, tag="sq", name="sq")
                    nc.vector.tensor_mul(
                        sq[:], qf[:, ch * 512:(ch + 1) * 512],
                        qf[:, ch * 512:(ch + 1) * 512])
                    nps = pp.tile([2, 512], f32, tag="pp", name="npp")
                    nc.tensor.matmul(nps[:], lhsT=msum_t[:], rhs=sq[:],
                                     start=True, stop=True)
                    rs = stg.tile([2, 512], f32, tag="rs", name="rs")
                    # 1/sqrt(s+eps) = exp(-0.5*ln(s+eps)); Ln+Exp share one
                    # activation table set (sqrt's is separate and would
                    # thrash), and beat sqrt's 65536-ULP budget.
                    nc.scalar.activation(rs[:], nps[:], AF.Ln, bias=eps_t[:])
                    nc.scalar.activation(
                        rcp[:, ch * 512:(ch + 1) * 512], rs[:], AF.Exp,
                        scale=-0.5)
                for ch in range(4):
                    bc = pp.tile([128, 512], f32, tag="pp", name="bc")
                    nc.tensor.matmul(
                        bc[:], lhsT=mbc_t[:],
                        rhs=rcp[:, ch * 512:(ch + 1) * 512],
                        start=True, stop=True,
                    )
                    nc.vector.tensor_mul(
                        dst[:, ch * 512:(ch + 1) * 512],
                        qf[:, ch * 512:(ch + 1) * 512], bc[:],
                    )

            exp_op = _register_exp_op() if "d" in EXP_CYCLE else None

            def exp_tile(e, s):
                eng = EXP_CYCLE[exp_idx[0] % len(EXP_CYCLE)]
                exp_idx[0] += 1
                if variant == "noexp":
                    nc.gpsimd.memset(e[:], 1.0)
                elif eng == "a":
                    nc.scalar.activation(e[:], s[:], AF.Exp)
                else:
                    nc.vector._custom_dve(
                        exp_op, out=e[:], in0=s[:],
                        s0=EXP_C0, s1=EXP_C1, imm2=EXP_C2,
                    )

            # ---- attention for one head (transposed PV) ----
            # pvt psum layout (f32 elems): isub k<7 at [65k, 65k+65),
            # k=7 at [512, 577) (the 65-wide accumulators must not cross the
            # 512-elem psum bank boundary); transpose scratch at [640, 768).
            def attend_t(h):
                mt, par = h // 2, h % 2
                pr = 64 * par
                for ic2 in range(2):
                    i0 = ic2 * 1024
                    pvt = pvp.tile([128, 1024], f32, tag="pv", name="pv")
                    nc.vector.memset(pvt[:, 0:455], 0.0)
                    nc.vector.memset(pvt[:, 512:577], 0.0)

                    def pvslice(k):
                        off = 65 * k if k < 7 else 512
                        return pvt[:, off:off + 65]

                    for jt in range(JT):
                        s = sp.tile([128, 1024], f32, tag="s", name="s")
                        for c in range(2):
                            nc.tensor.matmul(
                                s[:, c * 512:(c + 1) * 512],
                                lhsT=kn_t[mt][pr:pr + 64,
                                              jt * 128:(jt + 1) * 128],
                                rhs=qn_t[mt][pr:pr + 64,
                                             i0 + c * 512:i0 + (c + 1) * 512],
                                start=True, stop=True,
                            )
                        e = ep.tile([128, 1024], bf16, tag="e", name="e")
                        exp_tile(e, s)
                        for k in range(8):
                            nc.tensor.matmul(
                                pvslice(k),
                                lhsT=e[:, k * 128:(k + 1) * 128],
                                rhs=v4[:, jt, h, :],
                                start=False, stop=(jt == JT - 1),
                                skip_group_check=True,
                            )
                    # psum -> sbuf, per-token normalize on Pool, transpose
                    # back to [d, i] via PE, copy into outT
                    ot_sb = osb.tile([128, 520], f32, tag="osb", name="osb")
                    nc.vector.tensor_copy(ot_sb[:, 0:455], pvt[:, 0:455])
                    nc.vector.tensor_copy(ot_sb[:, 455:520], pvt[:, 512:577])
                    oti = osb.tile([128, 512], bf16, tag="oti", name="oti")
                    for k in range(8):
                        nc.gpsimd.normalize_recip(
                            oti[:, k * 64:(k + 1) * 64],
                            ot_sb[:, k * 65:k * 65 + 64],
                            ot_sb[:, k * 65 + 64:k * 65 + 65],
                        )
                    for k in range(8):
                        half = (k % 2) * 64
                        tpr = pvt[half:half + 64, 640:768].bitcast(bf16)
                        tpr = tpr[:, 0:128]
                        nc.tensor.transpose(
                            tpr, oti[:, k * 64:(k + 1) * 64], id128[:])
                        nc.vector.tensor_copy(
                            outT_t[mt][pr:pr + 64,
                                       i0 + k * 128:i0 + (k + 1) * 128],
                            tpr,
                        )

            # ---- attention for one head (flat PV, original scheme kept for
            # fallback benchmarking) ----
            ones_t = None

            def attend_flat(h):
                nonlocal ones_t
                if ones_t is None:
                    ones_t = cpool.tile([1, 64], f32, tag="ones1",
                                        name="ones1")
                    nc.vector.memset(ones_t[:], 1.0)
                mt, par = h // 2, h % 2
                pr = 64 * par
                for ic2 in range(2):
                    i0 = ic2 * 1024
                    pv = pvp.tile([128, 1024], f32, tag="pv", name="pv")
                    for jt in range(JT):
                        s = sp.tile([128, 1024], f32, tag="s", name="s")
                        for c in range(2):
                            nc.tensor.matmul(
                                s[:, c * 512:(c + 1) * 512],
                                lhsT=kn_t[mt][pr:pr + 64,
                                              jt * 128:(jt + 1) * 128],
                                rhs=qn_t[mt][pr:pr + 64,
                                             i0 + c * 512:i0 + (c + 1) * 512],
                                start=True, stop=True,
                            )
                        e = ep.tile([128, 1024], bf16, tag="e", name="e")
                        exp_tile(e, s)
                        for c in range(2):
                            nc.tensor.matmul(
                                pv[0:65, c * 512:(c + 1) * 512],
                                lhsT=v4[:, jt, h, :],
                                rhs=e[:, c * 512:(c + 1) * 512],
                                start=(jt == 0), stop=(jt == JT - 1),
                            )
                    rcd = stg.tile([1, 1024], f32, tag="rcd", name="rcd",
                                   bufs=1)
                    nc.vector.reciprocal(rcd[:], pv[64:65, 0:1024])
                    bc = sp.tile([64, 1024], f32, tag="s", name="bcd")
                    for c in range(2):
                        nc.tensor.matmul(
                            bc[:, c * 512:(c + 1) * 512],
                            lhsT=ones_t[:],
                            rhs=rcd[:, c * 512:(c + 1) * 512],
                            start=True, stop=True,
                        )
                    bcs = stg.tile([64, 1024], f32, tag="bcs", name="bcs",
                                   bufs=1)
                    nc.vector.tensor_copy(bcs[:], bc[:])
                    nc.vector.tensor_mul(
                        outT_t[mt][pr:pr + 64, i0:i0 + 1024], pv[0:64, 0:1024],
                        bcs[:]
                    )

            attend = attend_t if PV_TRANSPOSED else attend_flat

            # interleave: projections of pair mt feed heads 2mt, 2mt+1 while
            # the (ACT-heavy) attention of the previous pair still runs.
            for mt in range(MT):
                if variant in ("dmaonly", "c1"):
                    break
                qk_proj(mt, "q", qn_t[mt], mt)
                qk_proj(mt, "k", kn_t[mt], 4 + mt)
                attend(2 * mt)
                attend(2 * mt + 1)

            # ---- output projection, transposed: yT[eo, n] ----
            for et in range(1 if variant in ("dmaonly", "c1") else ET):
                yt = yp.tile([128, N], f32, tag="y", name="y")
                for ch in range(4):
                    ps = pp.tile([128, 512], f32, tag="pp", name="yps")
                    for mt in range(MT):
                        nc.tensor.matmul(
                            ps[:],
                            lhsT=wo_t[mt][:, et * 128:(et + 1) * 128],
                            rhs=outT_t[mt][:, ch * 512:(ch + 1) * 512],
                            start=(mt == 0), stop=(mt == MT - 1),
                        )
                    nc.vector.tensor_copy(yt[:, ch * 512:(ch + 1) * 512],
                                          ps[:])
                nc.sync.dma_start(y[:, et, :], yt[:])

    nc.finalize()
    _CACHE[key] = nc
    return nc


def make_in_maps(x, Wq_w, Wq_b, Wk_w, Wk_b, Wv_w, Wv_b, Wo_w, Wo_b):
    x = np.asarray(x, dtype=np.float32)
    wdt = FP8 if PROJ_FP8 else BF16
    ws = W_SCALE if PROJ_FP8 else 1.0

    def pmajor(a, tiles, dt):
        # [tiles*128, F] -> [128, tiles, F] (partition-major)
        return np.ascontiguousarray(
            a.reshape(tiles, 128, a.shape[1]).transpose(1, 0, 2)
        ).astype(dt)

    in_maps = []
    for c in range(M_CORES):
        b, g = c // 2, c % 2
        cols = slice(g * EC, (g + 1) * EC)
        qb = np.asarray(Wq_b, np.float32)[cols].reshape(MT, 128) * ws
        kb = np.asarray(Wk_b, np.float32)[cols].reshape(MT, 128) * ws
        qkb8 = np.zeros((8, 128), np.float32)
        qkb8[0:MT] = qb
        qkb8[4:4 + MT] = kb
        in_maps.append({
            "xT": pmajor(np.ascontiguousarray(x[b].T), ET, wdt),
            "wq": pmajor(np.asarray(Wq_w, np.float32)[:, cols] * ws, ET, wdt),
            "wk": pmajor(np.asarray(Wk_w, np.float32)[:, cols] * ws, ET, wdt),
            "wv": pmajor(np.asarray(Wv_w, np.float32)[:, cols] * ws, ET, wdt),
            "wo": pmajor(np.asarray(Wo_w, np.float32)[cols, :] / ws, MT, BF16),
            "qkb8": qkb8,
        })
    return in_maps


def assemble(results, Wv_b, Wo_w, Wo_b):
    bias_eff = (np.asarray(Wv_b, np.float32) @ np.asarray(Wo_w, np.float32)
                + np.asarray(Wo_b, np.float32))
    out = np.empty((B, N, E), np.float32)
    for b in range(B):
        # y is [128, ET, N] partition-major of yT [E, N]
        yT = results[2 * b]["y"] + results[2 * b + 1]["y"]
        yT = yT.transpose(1, 0, 2).reshape(E, N)
        out[b] = yT.T + bias_eff
    return out


def kernel(x, Wq_w, Wq_b, Wk_w, Wk_b, Wv_w, Wv_b, Wo_w, Wo_b):
    from concourse.bass_utils import run_bass_kernel_spmd

    nc = build_nc()
    in_maps = make_in_maps(x, Wq_w, Wq_b, Wk_w, Wk_b, Wv_w, Wv_b, Wo_w, Wo_b)
    res = run_bass_kernel_spmd(nc, in_maps, list(range(M_CORES)))
    return assemble(res.results, Wv_b, Wo_w, Wo_b)
